# revision 2
# baseline (speedup 1.0000x reference)
"""Trainium2 Bass kernel for a SAGAN-style attention block.

Per batch b:
    xf = x[b].reshape(C, N)                       # C=256, N=4096
    f = (wq / sigma(wq)) @ xf                     # [32, N]
    g = (wk / sigma(wk)) @ xf                     # [32, N]
    h = (wv / sigma(wv)) @ xf                     # [C, N]
    beta = softmax_over_rows(f.T @ g)             # [N, N], softmax over axis 0
    out = gamma * h @ beta + xf

Sharding: 8 cores = (batch b in 0..3) x (column half s in 0..1).  The
softmax normalizes each *column* of the score map over its rows, so a
column shard needs all of f and h but only its own columns of g / the
residual -- shards are fully independent, no cross-core communication.

Per-core kernel layout tricks:
  * scores are built in [n, m] layout (n on partitions) so exp() is a
    plain activation; the softmax denominator is obtained by appending a
    ones-column to h^T so the same accumulating matmul that computes
    (exp(s))^T @ h^T also emits the per-column sum as its 257th output
    column -- and the output lands transposed ([m, c], m on partitions),
    which turns the softmax division into a cheap per-partition
    tensor_scalar multiply.
  * spectral norms (tiny SVDs) + gamma folding are host-side weight prep.
"""

from contextlib import ExitStack

import numpy as np

import concourse.bass as bass
import concourse.tile as tile
from concourse import bacc, mybir
from concourse.bass_utils import run_bass_kernel_spmd

P = 128          # SBUF partitions
C = 256          # value channels
CO = 32          # query/key channels
N = 4096         # H*W sequence length
MS = 2048        # column shard width per core
NCH = N // P     # 32 row chunks of the score map
MTW = 512        # column tile width for the scores matmul
MT = MS // MTW   # 4 column tiles
MSUB = MS // P   # 16 column sub-tiles of 128
F32 = mybir.dt.float32
NCORES = 8

_ts = bass.ts


def _emit(tc: tile.TileContext, xf_d, xres_d, xresT_d, wqT_d, wkT_d, wvT_d, out_d):
    nc = tc.nc
    with ExitStack() as ctx:
        consts = ctx.enter_context(tc.tile_pool(name="consts", bufs=1))

        xf0 = consts.tile([P, N], F32)
        xf1 = consts.tile([P, N], F32)
        nc.sync.dma_start(xf0[:], xf_d[0])
        nc.sync.dma_start(xf1[:], xf_d[1])

        xres0 = consts.tile([P, MS], F32)
        xres1 = consts.tile([P, MS], F32)
        nc.sync.dma_start(xres0[:], xres_d[0])
        nc.sync.dma_start(xres1[:], xres_d[1])

        xresT_sb = consts.tile([P, MSUB, C], F32)
        for t in range(MSUB):
            nc.sync.dma_start(xresT_sb[:, t, :], xresT_d[t])

        wq0 = consts.tile([P, CO], F32)
        wq1 = consts.tile([P, CO], F32)
        wk0 = consts.tile([P, CO], F32)
        wk1 = consts.tile([P, CO], F32)
        wv0 = consts.tile([P, C], F32)
        wv1 = consts.tile([P, C], F32)
        nc.sync.dma_start(wq0[:], wqT_d[0])
        nc.sync.dma_start(wq1[:], wqT_d[1])
        nc.sync.dma_start(wk0[:], wkT_d[0])
        nc.sync.dma_start(wk1[:], wkT_d[1])
        nc.sync.dma_start(wv0[:], wvT_d[0])
        nc.sync.dma_start(wv1[:], wvT_d[1])

        f_sb = consts.tile([CO, N], F32)
        g_sb = consts.tile([CO, MS], F32)
        # h^T with a ones-column appended per row chunk: [n, c0..c255, 1.0]
        hT_sb = consts.tile([P, NCH, C + 1], F32)
        for k in range(NCH):
            nc.vector.memset(hT_sb[:, k, C : C + 1], 1.0)

        with tc.tile_pool(name="pro_ps", bufs=2, space="PSUM") as pro_ps:
            # f = wqn @ xf : [CO, N]
            for t in range(N // MTW):
                ps = pro_ps.tile([CO, MTW], F32, tag="fg")
                nc.tensor.matmul(ps[:], wq0[:], xf0[:, _ts(t, MTW)], start=True, stop=False)
                nc.tensor.matmul(ps[:], wq1[:], xf1[:, _ts(t, MTW)], start=False, stop=True)
                nc.vector.tensor_copy(f_sb[:, _ts(t, MTW)], ps[:])
            # g = wkn @ xres : [CO, MS]
            for t in range(MT):
                ps = pro_ps.tile([CO, MTW], F32, tag="fg")
                nc.tensor.matmul(ps[:], wk0[:], xres0[:, _ts(t, MTW)], start=True, stop=False)
                nc.tensor.matmul(ps[:], wk1[:], xres1[:, _ts(t, MTW)], start=False, stop=True)
                nc.vector.tensor_copy(g_sb[:, _ts(t, MTW)], ps[:])
            # hT[n, c] = sum_c' xf[c', n] * wvT[c', c]  (gamma folded into wvT)
            for k in range(NCH):
                ps = pro_ps.tile([P, C], F32, tag="h")
                nc.tensor.matmul(ps[:], xf0[:, _ts(k, P)], wv0[:], start=True, stop=False)
                nc.tensor.matmul(ps[:], xf1[:, _ts(k, P)], wv1[:], start=False, stop=True)
                nc.vector.tensor_copy(hT_sb[:, k, :C], ps[:])

        with (
            tc.tile_pool(name="sc_ps", bufs=4, space="PSUM") as sc_ps,
            tc.tile_pool(name="acc_ps", bufs=4, space="PSUM") as acc_ps,
            tc.tile_pool(name="work", bufs=4) as work,
            tc.tile_pool(name="outp", bufs=4) as outp,
        ):
            for mt in range(MT):
                accs = [
                    acc_ps.tile([P, C + 1], F32, tag="acc", name=f"acc_{mt}_{sub}")
                    for sub in range(4)
                ]
                for k in range(NCH):
                    # scores[n_chunk, m_tile] = f[:, chunk].T @ g[:, m_tile]
                    sps = sc_ps.tile([P, MTW], F32, tag="s", name=f"s_{mt}_{k}")
                    nc.tensor.matmul(
                        sps[:], f_sb[:, _ts(k, P)], g_sb[:, _ts(mt, MTW)],
                        start=True, stop=True,
                    )
                    et = work.tile([P, MTW], F32, tag="e", name=f"e_{mt}_{k}")
                    nc.scalar.activation(et[:], sps[:], mybir.ActivationFunctionType.Exp)
                    # acc[m_sub, c | colsum] += exp(s)[:, sub].T @ hT_aug[chunk]
                    for sub in range(4):
                        nc.tensor.matmul(
                            accs[sub][:], et[:, _ts(sub, P)], hT_sb[:, k, :],
                            start=(k == 0), stop=(k == NCH - 1),
                        )
                for sub in range(4):
                    mi = mt * 4 + sub
                    rec = work.tile([P, 1], F32, tag="r", name=f"r_{mi}")
                    nc.vector.reciprocal(rec[:], accs[sub][:, C : C + 1])
                    ot = outp.tile([P, C], F32, tag="o", name=f"o_{mi}")
                    nc.vector.tensor_scalar_mul(ot[:], accs[sub][:, :C], rec[:])
                    ot2 = outp.tile([P, C], F32, tag="o2", name=f"o2_{mi}")
                    nc.vector.tensor_add(ot2[:], ot[:], xresT_sb[:, mi, :])
                    nc.sync.dma_start(out_d[mi], ot2[:])


def build_program():
    nc = bacc.Bacc("TRN2", target_bir_lowering=False, debug=False, num_devices=NCORES)
    xf_d = nc.dram_tensor("xf", [2, P, N], F32, kind="ExternalInput")
    xres_d = nc.dram_tensor("xres", [2, P, MS], F32, kind="ExternalInput")
    xresT_d = nc.dram_tensor("xresT", [MSUB, P, C], F32, kind="ExternalInput")
    wqT_d = nc.dram_tensor("wqT", [2, P, CO], F32, kind="ExternalInput")
    wkT_d = nc.dram_tensor("wkT", [2, P, CO], F32, kind="ExternalInput")
    wvT_d = nc.dram_tensor("wvT", [2, P, C], F32, kind="ExternalInput")
    out_d = nc.dram_tensor("out", [MSUB, P, C], F32, kind="ExternalOutput")
    with tile.TileContext(nc) as tc:
        _emit(tc, xf_d, xres_d, xresT_d, wqT_d, wkT_d, wvT_d, out_d)
    nc.compile()
    return nc


_PROGRAM = None


def _get_program():
    global _PROGRAM
    if _PROGRAM is None:
        _PROGRAM = build_program()
    return _PROGRAM


def make_in_maps(x, w_q, w_k, w_v, gamma):
    x = np.ascontiguousarray(x, dtype=np.float32)
    wqn = (w_q / np.linalg.norm(w_q, 2)).astype(np.float32)
    wkn = (w_k / np.linalg.norm(w_k, 2)).astype(np.float32)
    wvg = (np.float32(gamma[0]) * (w_v / np.linalg.norm(w_v, 2))).astype(np.float32)
    wqT = np.ascontiguousarray(wqn.T).reshape(2, P, CO)
    wkT = np.ascontiguousarray(wkn.T).reshape(2, P, CO)
    wvT = np.ascontiguousarray(wvg.T).reshape(2, P, C)
    B = x.shape[0]
    xf = x.reshape(B, C, N)
    in_maps = []
    for core in range(NCORES):
        b, s = divmod(core, 2)
        xb = xf[b]
        xres = np.ascontiguousarray(xb[:, s * MS : (s + 1) * MS])
        in_maps.append(
            {
                "xf": np.ascontiguousarray(xb).reshape(2, P, N),
                "xres": xres.reshape(2, P, MS),
                "xresT": np.ascontiguousarray(xres.T).reshape(MSUB, P, C),
                "wqT": wqT,
                "wkT": wkT,
                "wvT": wvT,
            }
        )
    return in_maps


def assemble_output(results, x_shape):
    B, _, H, W = x_shape
    out = np.empty((B, C, N), np.float32)
    for core in range(NCORES):
        b, s = divmod(core, 2)
        oT = np.asarray(results[core]["out"]).reshape(MS, C)  # [m, c]
        out[b, :, s * MS : (s + 1) * MS] = oT.T
    return out.reshape(B, C, H, W)


def run(x, w_q, w_k, w_v, gamma, trace=False, **kwargs):
    nc = _get_program()
    in_maps = make_in_maps(x, w_q, w_k, w_v, gamma)
    res = run_bass_kernel_spmd(nc, in_maps, list(range(NCORES)), trace=trace, **kwargs)
    return assemble_output(res.results, x.shape), res


def kernel(x, w_q, w_k, w_v, gamma):
    out, _ = run(
        np.asarray(x), np.asarray(w_q), np.asarray(w_k),
        np.asarray(w_v), np.asarray(gamma),
    )
    return out
